# revision 1
# baseline (speedup 1.0000x reference)
"""Trainium2 Bass kernel for the differentiable-Kalman-filter loss.

Math: the reference runs a T=100000-step linear recurrence
  x_{i+1} = M x_i + K obs[i-1],  eps_i = obs[i] - C x_{i+1},  M = A - K C
and accumulates yvar = sum outer(eps_i) + decaying P-terms, loss = slogdet(yvar/T).
rho(M) ~ 0.963, so the recurrence has ~400-step memory: eps becomes a truncated
causal convolution of obs.  Each core computes eps for a 12160-row slab via a
two-level blocked conv (B=16 within-block taps as one 512x512 triangular matmul,
block-boundary states from J=24 block-level taps), then accumulates the Gram
E^T E on-chip.  The first W=2720 rows + the tiny P-series are computed exactly
on host in f64 (they need the exact initial transient and cost ~nothing).
"""
import numpy as np

T, N, B, J, W, NCORES = 100000, 32, 16, 16, 2720, 8
R = (T - W) // NCORES       # rows per core = 12160
NB = R // B                 # 760 blocks per core
PSI = NB + J                # 784 panel columns (incl halo)
NTS = 6                     # s-tiles per core
PS = [128, 128, 128, 128, 128, 120]
J0S = [0, 4, 8, 12]

_PROG_CACHE = {}


def _build_device_consts(A64, C64, K64):
    import ml_dtypes
    bf16 = ml_dtypes.bfloat16
    M = A64 - K64 @ C64
    Mp = [np.eye(N)]
    for _ in range(B + 1):
        Mp.append(M @ Mp[-1])
    H = [C64 @ Mp[k] @ K64 for k in range(B)]
    TrilHneg = np.zeros((512, 512))
    for r in range(B):
        for t in range(r, B):
            TrilHneg[r*N:(r+1)*N, t*N:(t+1)*N] = -H[t - r].T
    Gmat = np.zeros((512, N))
    for r in range(B):
        Gmat[r*N:(r+1)*N, :] = (Mp[B-1-r] @ K64).T
    CMn = np.zeros((N, 512))
    for t in range(B):
        CMn[:, t*N:(t+1)*N] = -(C64 @ Mp[t+1]).T
    MB = Mp[B]
    D = [np.eye(N)]
    for _ in range(J - 1):
        D.append(MB @ D[-1])
    trilh = np.ascontiguousarray(TrilHneg.reshape(4, 128, 512).transpose(1, 0, 2).reshape(128, 2048)).astype(bf16)
    gmat = np.ascontiguousarray(Gmat.reshape(4, 128, N).transpose(1, 0, 2).reshape(128, 128)).astype(bf16)
    cmn = np.ascontiguousarray(CMn).astype(bf16)
    dstk = np.zeros((128, 32 * (J // 4)))
    for jg in range(J // 4):
        for rho in range(4):
            dstk[32*rho:32*rho+32, 32*jg:32*jg+32] = D[4*jg + rho].T
    dstk = dstk.astype(bf16)
    identb = np.eye(32).astype(bf16)
    return trilh, gmat, cmn, dstk, identb


def _host_exact(obs, A64, C64, K64, x0, Psqrt0):
    """f64 exact: P-series + outer(obs0) + eps outers for i < W."""
    obs64 = obs.astype(np.float64)
    M = A64 - K64 @ C64
    Y = np.outer(obs64[0], obs64[0])
    P = Psqrt0.astype(np.float64)
    for _ in range(4000):
        CP = C64 @ P
        Y += CP @ CP.T
        P = M @ P
        if np.abs(P).max() < 1e-16:
            break
    x = x0.astype(np.float64)
    for i in range(W):
        o_prev = obs64[i - 1] if i > 0 else obs64[T - 1]
        x = M @ x + K64 @ o_prev
        eps = obs64[i] - C64 @ x
        Y += np.outer(eps, eps)
    return Y


def _patch_tile_drain():
    """This walrus build allows only one sem wait per Drain; split the
    TileContext tail drain's waits across multiple drain instructions."""
    import concourse.tile as tile
    from concourse.vector_clock import ScopedClock
    if getattr(tile.TileContext, "_kf_drain_patched", False):
        return
    def _drain_and_barrier(self, tick_clock, wait_clock):
        nc = self.nc
        drain_inst = nc.sync.drain()
        wait_clock.add_sem_waits(drain_inst.ins, ScopedClock({None: tick_clock.global_clock}))
        si = drain_inst.ins.sync_info
        waits = list(si.on_wait or [])
        if len(waits) > 1:
            si.on_wait = waits[:1]
            for i in range(1, len(waits)):
                extra = nc.sync.drain()
                esi = extra.ins.sync_info
                if esi is None:
                    extra.ins.sync_info = type(si)(on_wait=waits[i:i+1], on_update=[])
                else:
                    esi.on_wait = waits[i:i+1]
        nc.all_engine_barrier(sem_only=True)
        assert self.sems is not None
        popped = nc._tile_sem_poison_stack.pop()
        assert popped is self._sem_poison
        nc.clear_and_free_semaphores(list(self.sems.allocated().values()))
    tile.TileContext._drain_and_barrier = _drain_and_barrier
    tile.TileContext._kf_drain_patched = True


def _split_multi_waits(nc):
    """This walrus build encodes at most one sem wait per instruction; hoist
    extra waits onto NoOps inserted just before in the same engine stream."""
    import concourse.mybir as mybir
    for func in nc.m.functions:
        for blk in func.blocks:
            insts = blk.instructions
            out, changed = [], False
            for inst in insts:
                si = inst.sync_info
                waits = list(si.on_wait) if si and si.on_wait else []
                if len(waits) > 1:
                    changed = True
                    for k, w in enumerate(waits[:-1]):
                        out.append(mybir.InstNoOp(
                            name=f"{inst.name}-hw{k}", engine=inst.engine,
                            bass_nofuse=True,
                            sync_info=mybir.SyncInfo(on_wait=[w], on_update=[])))
                    si.on_wait = [waits[-1]]
                out.append(inst)
            if changed:
                blk.instructions = out


def build_program(debug=False):
    import concourse.bass as bass
    import concourse.mybir as mybir
    import concourse.tile as tile
    _patch_tile_drain()
    f32 = mybir.dt.float32
    bf16 = mybir.dt.bfloat16

    nc = bass.Bass()
    # host-prepped bf16 inputs: obsb = Oblk tiles, pans = transposed panels (+halo)
    obsb_in = nc.declare_dram_parameter("obsb", [128, 6 * 512], bf16, isOutput=False)
    pans_in = nc.declare_dram_parameter("pans", [128, 4 * 768 + 4 * 32], bf16, isOutput=False)
    trilh_in = nc.declare_dram_parameter("trilh", [128, 2048], bf16, isOutput=False)
    gmat_in = nc.declare_dram_parameter("gmat", [128, 128], bf16, isOutput=False)
    cmn_in = nc.declare_dram_parameter("cmn", [32, 512], bf16, isOutput=False)
    dstk_in = nc.declare_dram_parameter("dstk", [128, 32 * (J // 4)], bf16, isOutput=False)
    identb_in = nc.declare_dram_parameter("identb", [32, 32], bf16, isOutput=False)
    yout = nc.declare_dram_parameter("yout", [128, 512], f32, isOutput=True)
    if debug:
        dbg_gt = nc.declare_dram_parameter("dbg_gt", [32, PSI], f32, isOutput=True)
        dbg_xbt = nc.declare_dram_parameter("dbg_xbt", [32, NB], f32, isOutput=True)
        dbg_e0 = nc.declare_dram_parameter("dbg_e0", [128, 512], f32, isOutput=True)

    HAL = 4 * 768   # halo column offset inside pans

    with tile.TileContext(nc) as tc:
        with (
            tc.tile_pool(name="consts", bufs=1) as cpool,
            tc.tile_pool(name="obs", bufs=1) as opool,
            tc.tile_pool(name="work", bufs=1) as wpool,
            tc.tile_pool(name="etile", bufs=3) as epool,
            tc.tile_pool(name="trps", bufs=2, space="PSUM") as trpool,
            tc.tile_pool(name="epsum", bufs=3, space="PSUM") as eppool,
            tc.tile_pool(name="gramps", bufs=1, space="PSUM") as gpool,
        ):
            # ---- inputs: two big DMAs on separate HWDGE rings, consts via SWDGE
            pans = opool.tile([128, 4 * 768 + 4 * 32], bf16)
            obsb = opool.tile([128, 6 * 512], bf16)
            gmat = cpool.tile([128, 128], bf16)
            cmn = cpool.tile([32, 512], bf16)
            dstk = cpool.tile([128, 32 * (J // 4)], bf16)
            identb = cpool.tile([32, 32], bf16)
            trilh = cpool.tile([128, 2048], bf16)
            # sync ring: halo+gmat (first PE consumers), panel chunks, trilh half
            nc.sync.dma_start(pans[:, HAL : HAL+128], pans_in[:, HAL : HAL+128])
            nc.sync.dma_start(gmat[:], gmat_in[:])
            for kc in range(4):
                nc.sync.dma_start(pans[:, kc*768 : (kc+1)*768],
                                  pans_in[:, kc*768 : (kc+1)*768])
            nc.sync.dma_start(trilh[:, 0:1024], trilh_in[:, 0:1024])
            # scalar ring: small consts, obsb, trilh half
            nc.scalar.dma_start(identb[:], identb_in[:])
            nc.scalar.dma_start(cmn[:], cmn_in[:])
            nc.scalar.dma_start(dstk[:], dstk_in[:])
            nc.scalar.dma_start(trilh[:, 1024:2048], trilh_in[:, 1024:2048])
            nc.scalar.dma_start(obsb[:, 0:1536], obsb_in[:, 0:1536])
            nc.scalar.dma_start(obsb[:, 1536:3072], obsb_in[:, 1536:3072])

            # ---- PE warmup: junk matmuls with no input deps run during the
            # input-DMA wait and lift the HAM clock gate to 2.4 GHz before
            # real matmuls start.
            warm = eppool.tile([128, 512], f32, tag="epsum")
            for _ in range(24):
                nc.tensor.matmul(warm[:, 0:128],
                                 lhsT=pans[:, HAL : HAL+128],
                                 rhs=pans[:, HAL : HAL+128],
                                 start=True, stop=True, skip_group_check=True)

            # ---- gT [32, 784]: halo part + main part
            gth_ps = trpool.tile([32, 32], f32, tag="trps")
            for kc in range(4):
                nc.tensor.matmul(gth_ps[:, 0:J],
                                 lhsT=gmat[:, 32*kc : 32*kc+32],
                                 rhs=pans[:, HAL + 32*kc : HAL + 32*kc+J],
                                 start=(kc == 0), stop=(kc == 3))
            gtm_ps = trpool.tile([32, NB], f32, tag="trps")
            for c0, nn_ in [(0, 512), (512, NB - 512)]:
                for kc in range(4):
                    nc.tensor.matmul(gtm_ps[:, c0 : c0+nn_],
                                     lhsT=gmat[:, 32*kc : 32*kc+32],
                                     rhs=pans[:, kc*768 + c0 : kc*768 + c0 + nn_],
                                     start=(kc == 0), stop=(kc == 3))
            gts = wpool.tile([32, PSI], bf16)
            nc.vector.tensor_copy(gts[:, 0:J], gth_ps[:, 0:J])
            nc.vector.tensor_copy(gts[:, J:PSI], gtm_ps[:])

            # ---- gS [128, PSI]: group rho = gT shifted right by rho cols
            gs_ps = trpool.tile([128, PSI], f32, tag="trps")
            for rho in range(4):
                tp = (0, 32 * rho) if rho else None
                for c0, c1 in [(rho, 512), (512, PSI)]:
                    nc.tensor.matmul(gs_ps[32*rho : 32*rho+32, c0:c1],
                                     lhsT=identb[:],
                                     rhs=gts[:, c0-rho : c1-rho],
                                     start=True, stop=True, tile_position=tp)
            gss = wpool.tile([128, PSI], bf16)
            nc.vector.tensor_copy(gss[:], gs_ps[:])

            # ---- XbT [32, 760]: sum_j D_j g_{s+23-j} via 6 tap-groups of 4
            xbt_ps = trpool.tile([32, NB], f32, tag="trps")
            for jg, j0 in enumerate(J0S):
                for c0, nn_ in [(0, 512), (512, NB - 512)]:
                    nc.tensor.matmul(xbt_ps[:, c0 : c0+nn_],
                                     lhsT=dstk[:, 32*jg : 32*jg+32],
                                     rhs=gss[:, (J-1-j0)+c0 : (J-1-j0)+c0+nn_],
                                     start=(j0 == 0), stop=(j0 == J0S[-1]))
            xbt = wpool.tile([32, NB], bf16)
            nc.vector.tensor_copy(xbt[:], xbt_ps[:])

            # ---- conv + E + Gram
            gram_ps = gpool.tile([128, 512], f32)
            for st in range(NTS):
                p = PS[st]
                eps_ps = eppool.tile([128, 512], f32, tag="epsum")
                for kc in range(4):
                    nc.tensor.matmul(eps_ps[:p, :],
                                     lhsT=pans[:, kc*768 + 128*st : kc*768 + 128*st + p],
                                     rhs=trilh[:, 512*kc : 512*kc+512],
                                     start=(kc == 0), stop=False)
                nc.tensor.matmul(eps_ps[:p, :],
                                 lhsT=xbt[:, 128*st : 128*st+p],
                                 rhs=cmn[:, :],
                                 start=False, stop=True)
                esb = epool.tile([128, 512], bf16, tag="etile")
                nc.vector.tensor_add(esb[:p, :],
                                     obsb[:p, 512*st : 512*st+512],
                                     eps_ps[:p, :])
                if debug and st == 0:
                    nc.sync.dma_start(dbg_e0[:], esb[:])
                for g in range(4):
                    # start=True zeroes the full 2KB bank row per written
                    # partition, so only the very first matmul may set it.
                    nc.tensor.matmul(gram_ps[:, 128*g : 128*g+128],
                                     lhsT=esb[:p, 128*g : 128*g+128],
                                     rhs=esb[:p, 128*g : 128*g+128],
                                     start=(st == 0 and g == 0),
                                     stop=(st == NTS - 1 and g == 3),
                                     skip_group_check=True)

            ysb = wpool.tile([128, 512], f32)
            nc.vector.tensor_copy(ysb[:], gram_ps[:])
            nc.sync.dma_start(yout[:], ysb[:])
            if debug:
                nc.sync.dma_start(dbg_gt[:], gts[:])
                nc.sync.dma_start(dbg_xbt[:], xbt[:])

    _split_multi_waits(nc)
    return nc


def _core_inputs(obs, c, consts):
    """Host-side layout prep for one core: bf16 Oblk tiles + transposed panels."""
    import ml_dtypes
    bf16 = ml_dtypes.bfloat16
    trilh, gmat, cmn, dstk, identb = consts
    start = W + c * R
    hb = J * B + 1                                      # halo rows + 1
    flat = obs[start - hb : start + R]
    # Oblk tiles: rows [start+16s, +16) for s in [0, 760)
    ob = np.zeros((768, 512), np.float32)
    ob[:NB] = flat[hb : hb + R].reshape(NB, 512)
    obsb = np.ascontiguousarray(
        ob.reshape(6, 128, 512).transpose(1, 0, 2).reshape(128, 6 * 512)).astype(bf16)
    # panel rows (shifted by -1 obs row): s in [0, 760)
    pm = np.zeros((768, 512), np.float32)
    pm[:NB] = flat[hb - 1 : hb - 1 + R].reshape(NB, 512)
    ptm = pm.reshape(768, 4, 128).transpose(2, 1, 0)    # [128, 4, 768]
    pth = np.zeros((128, 4, 32), np.float32)
    ph = flat[0 : J * B].reshape(J, 512)                # halo panel rows
    pth[:, :, :J] = ph.reshape(J, 4, 128).transpose(2, 1, 0)
    pans = np.ascontiguousarray(np.concatenate(
        [ptm.reshape(128, 4 * 768), pth.reshape(128, 4 * 32)], axis=1)).astype(bf16)
    return {"obsb": obsb, "pans": pans, "trilh": trilh, "gmat": gmat,
            "cmn": cmn, "dstk": dstk, "identb": identb}


def kernel(observations, A, C, K, x0, Psqrt0, _trace=False, _trace_kwargs=None):
    obs = np.ascontiguousarray(observations, np.float32)
    A64 = np.asarray(A, np.float64)
    C64 = np.asarray(C, np.float64)
    K64 = np.asarray(K, np.float64)

    consts = _build_device_consts(A64, C64, K64)
    Y = _host_exact(obs, A64, C64, K64, np.asarray(x0), np.asarray(Psqrt0))

    if "prog" not in _PROG_CACHE:
        _PROG_CACHE["prog"] = build_program()
    nc = _PROG_CACHE["prog"]

    in_maps = [_core_inputs(obs, c, consts) for c in range(NCORES)]

    from concourse.bass_utils import run_bass_kernel_spmd
    kw = dict(_trace_kwargs or {})
    res = run_bass_kernel_spmd(nc, in_maps, list(range(NCORES)), trace=_trace, **kw)

    for c in range(NCORES):
        G = np.asarray(res.results[c]["yout"], np.float64)
        for g in range(4):
            for tau in range(4):
                Y += G[32*tau:32*tau+32, 128*g+32*tau : 128*g+32*tau+32]
    loss = np.linalg.slogdet(Y / T)[1]
    out = np.float32(loss)
    if _trace:
        return out, res
    return out



# revision 4
# speedup vs baseline: 1.1029x; 1.1029x over previous
"""Trainium2 Bass kernel for the differentiable-Kalman-filter loss.

Math: the reference runs a T=100000-step linear recurrence
  x_{i+1} = M x_i + K obs[i-1],  eps_i = obs[i] - C x_{i+1},  M = A - K C
and accumulates yvar = sum outer(eps_i) + decaying P-terms, loss = slogdet(yvar/T).
rho(M) ~ 0.963, so the recurrence has ~400-step memory: eps becomes a truncated
causal convolution of obs.  Each core computes eps for a 12160-row slab via a
two-level blocked conv (B=16 within-block taps as one 512x512 triangular matmul,
block-boundary states from J=16 block-level taps), then accumulates the Gram
E^T E on-chip.  The first W=2720 rows + the tiny P-series are computed exactly
on host in f64 (they need the exact initial transient and cost ~nothing).

Perf layout (v2): inputs packed into 3 DRAM params and moved with 5 big
need-ordered DMAs (HWDGE ~600ns issue cost each, so few+large wins); PE kept
busy from ~7us via junk matmuls on a memset tile (HAM clock gate); late eps
groups start with the xbt term so the last tile's tail is short.
"""
import numpy as np

T, N, B, J, W, NCORES = 100000, 32, 16, 16, 2720, 8
R = (T - W) // NCORES       # rows per core = 12160
NB = R // B                 # 760 blocks per core
PSI = NB + J                # 776 panel columns (incl halo)
NTS = 6                     # s-tiles per core
PS = [128, 128, 128, 128, 128, 120]
J0S = [0, 4, 8, 12]

# bulkA column offsets: gmat | halo | pans | obsb-hi
A_HALO, A_PANS, A_OBH, A_COLS = 128, 256, 3328, 4864
# bulkB column offsets: trilh | dstk | obsb-lo
B_DSTK, B_OBL, B_COLS = 2048, 2176, 3712
S_COLS = 544                # smalls: identb(32) | cmn(512)

_PROG_CACHE = {}


def _build_device_consts(A64, C64, K64):
    import ml_dtypes
    bf16 = ml_dtypes.bfloat16
    M = A64 - K64 @ C64
    Mp = [np.eye(N)]
    for _ in range(B + 1):
        Mp.append(M @ Mp[-1])
    H = [C64 @ Mp[k] @ K64 for k in range(B)]
    TrilHneg = np.zeros((512, 512))
    for r in range(B):
        for t in range(r, B):
            TrilHneg[r*N:(r+1)*N, t*N:(t+1)*N] = -H[t - r].T
    Gmat = np.zeros((512, N))
    for r in range(B):
        Gmat[r*N:(r+1)*N, :] = (Mp[B-1-r] @ K64).T
    CMn = np.zeros((N, 512))
    for t in range(B):
        CMn[:, t*N:(t+1)*N] = -(C64 @ Mp[t+1]).T
    MB = Mp[B]
    D = [np.eye(N)]
    for _ in range(J - 1):
        D.append(MB @ D[-1])
    trilh = np.ascontiguousarray(TrilHneg.reshape(4, 128, 512).transpose(1, 0, 2).reshape(128, 2048)).astype(bf16)
    gmat = np.ascontiguousarray(Gmat.reshape(4, 128, N).transpose(1, 0, 2).reshape(128, 128)).astype(bf16)
    cmn = np.ascontiguousarray(CMn).astype(bf16)
    dstk = np.zeros((128, 32 * (J // 4)))
    for jg in range(J // 4):
        for rho in range(4):
            dstk[32*rho:32*rho+32, 32*jg:32*jg+32] = D[4*jg + rho].T
    dstk = dstk.astype(bf16)
    identb = np.eye(32).astype(bf16)
    return trilh, gmat, cmn, dstk, identb


def _host_exact(obs, A64, C64, K64, x0, Psqrt0):
    """f64 exact: P-series + outer(obs0) + eps outers for i < W."""
    obs64 = obs.astype(np.float64)
    M = A64 - K64 @ C64
    Y = np.outer(obs64[0], obs64[0])
    P = Psqrt0.astype(np.float64)
    for _ in range(4000):
        CP = C64 @ P
        Y += CP @ CP.T
        P = M @ P
        if np.abs(P).max() < 1e-16:
            break
    x = x0.astype(np.float64)
    for i in range(W):
        o_prev = obs64[i - 1] if i > 0 else obs64[T - 1]
        x = M @ x + K64 @ o_prev
        eps = obs64[i] - C64 @ x
        Y += np.outer(eps, eps)
    return Y


def _patch_tile_drain():
    """This walrus build allows only one sem wait per Drain; split the
    TileContext tail drain's waits across multiple drain instructions."""
    import concourse.tile as tile
    from concourse.vector_clock import ScopedClock
    if getattr(tile.TileContext, "_kf_drain_patched", False):
        return
    def _drain_and_barrier(self, tick_clock, wait_clock):
        nc = self.nc
        drain_inst = nc.sync.drain()
        wait_clock.add_sem_waits(drain_inst.ins, ScopedClock({None: tick_clock.global_clock}))
        si = drain_inst.ins.sync_info
        waits = list(si.on_wait or [])
        if len(waits) > 1:
            si.on_wait = waits[:1]
            for i in range(1, len(waits)):
                extra = nc.sync.drain()
                esi = extra.ins.sync_info
                if esi is None:
                    extra.ins.sync_info = type(si)(on_wait=waits[i:i+1], on_update=[])
                else:
                    esi.on_wait = waits[i:i+1]
        nc.all_engine_barrier(sem_only=True)
        assert self.sems is not None
        popped = nc._tile_sem_poison_stack.pop()
        assert popped is self._sem_poison
        nc.clear_and_free_semaphores(list(self.sems.allocated().values()))
    tile.TileContext._drain_and_barrier = _drain_and_barrier
    tile.TileContext._kf_drain_patched = True


def _split_multi_waits(nc):
    """This walrus build encodes at most one sem wait per instruction; hoist
    extra waits onto NoOps inserted just before in the same engine stream."""
    import concourse.mybir as mybir
    for func in nc.m.functions:
        for blk in func.blocks:
            insts = blk.instructions
            out, changed = [], False
            for inst in insts:
                si = inst.sync_info
                waits = list(si.on_wait) if si and si.on_wait else []
                if len(waits) > 1:
                    changed = True
                    for k, w in enumerate(waits[:-1]):
                        out.append(mybir.InstNoOp(
                            name=f"{inst.name}-hw{k}", engine=inst.engine,
                            bass_nofuse=True,
                            sync_info=mybir.SyncInfo(on_wait=[w], on_update=[])))
                    si.on_wait = [waits[-1]]
                out.append(inst)
            if changed:
                blk.instructions = out


def build_program(debug=False):
    import concourse.bass as bass
    import concourse.mybir as mybir
    import concourse.tile as tile
    _patch_tile_drain()
    f32 = mybir.dt.float32
    bf16 = mybir.dt.bfloat16

    nc = bass.Bass()
    A_in = nc.declare_dram_parameter("bulka", [128, A_COLS], bf16, isOutput=False)
    B_in = nc.declare_dram_parameter("bulkb", [128, B_COLS], bf16, isOutput=False)
    S_in = nc.declare_dram_parameter("smalls", [32, S_COLS], bf16, isOutput=False)
    yout = nc.declare_dram_parameter("yout", [128, 512], f32, isOutput=True)

    with tile.TileContext(nc) as tc:
        with (
            tc.tile_pool(name="big", bufs=1) as bpool,
            tc.tile_pool(name="work", bufs=1) as wpool,
            tc.tile_pool(name="etile", bufs=6) as epool,
            tc.tile_pool(name="ps2k", bufs=2, space="PSUM") as ppool,
            tc.tile_pool(name="epsum", bufs=4, space="PSUM") as eppool,
            tc.tile_pool(name="gramps", bufs=1, space="PSUM") as gpool,
        ):
            A = bpool.tile([128, A_COLS], bf16)
            Bt = bpool.tile([128, B_COLS], bf16)
            S = wpool.tile([32, S_COLS], bf16)
            warm = wpool.tile([128, 128], bf16)

            # ---- warm tile via memset (no DMA dep), smalls on SWDGE ring
            nc.gpsimd.memset(warm[:], 0.0)
            nc.gpsimd.dma_start(S[:], S_in[:])
            # ---- bulk inputs: few, big, need-ordered HWDGE DMAs
            nc.sync.dma_start(A[:, 0:A_PANS], A_in[:, 0:A_PANS])          # gmat+halo
            nc.sync.dma_start(A[:, A_PANS:A_OBH], A_in[:, A_PANS:A_OBH])  # pans
            nc.scalar.dma_start(Bt[:, 0:B_OBL], B_in[:, 0:B_OBL])         # trilh+dstk
            nc.scalar.dma_start(Bt[:, B_OBL:B_COLS], B_in[:, B_OBL:B_COLS])  # obsb st0-2
            nc.sync.dma_start(A[:, A_OBH:A_COLS], A_in[:, A_OBH:A_COLS])  # obsb st3-5

            gmat = A[:, 0:128]
            identb = S[:, 0:32]
            cmn = S[:, 32:544]

            gram_ps = gpool.tile([128, 512], f32)

            def junk(n):
                # PE keep-warm: HAM un-throttles only under sustained activity
                for _ in range(n):
                    nc.tensor.matmul(gram_ps[:, 0:128], lhsT=warm[:], rhs=warm[:],
                                     start=True, stop=True, skip_group_check=True)

            junk(16)

            # ---- gT [32, 776]: halo states + main panel states (split 512|264)
            gtA = ppool.tile([32, 512], f32, tag="ps2k")
            gtB = ppool.tile([32, 264], f32, tag="ps2k")
            for kc in range(4):
                nc.tensor.matmul(gtA[:, 0:J],
                                 lhsT=gmat[:, 32*kc : 32*kc+32],
                                 rhs=A[:, A_HALO + 32*kc : A_HALO + 32*kc + J],
                                 start=(kc == 0), stop=False)
            junk(10)
            for kc in range(4):
                nc.tensor.matmul(gtA[:, J : 512],
                                 lhsT=gmat[:, 32*kc : 32*kc+32],
                                 rhs=A[:, A_PANS + kc*768 : A_PANS + kc*768 + (512 - J)],
                                 start=False, stop=(kc == 3))
            for kc in range(4):
                nc.tensor.matmul(gtB[:, 0:264],
                                 lhsT=gmat[:, 32*kc : 32*kc+32],
                                 rhs=A[:, A_PANS + kc*768 + (512-J) : A_PANS + kc*768 + 760],
                                 start=(kc == 0), stop=(kc == 3))

            # ---- conv st0 (independent of the gT chain)
            eps_list = [None] * NTS
            esb_list = [None] * NTS

            def conv_st(st, start):
                p = PS[st]
                eps_ps = eps_list[st]
                for kc in range(4):
                    nc.tensor.matmul(eps_ps[:p, :],
                                     lhsT=A[:, A_PANS + kc*768 + 128*st : A_PANS + kc*768 + 128*st + p],
                                     rhs=Bt[:, 512*kc : 512*kc+512],
                                     start=(start and kc == 0),
                                     stop=(not start and kc == 3))

            def xcmn_st(st, start, xbt):
                p = PS[st]
                nc.tensor.matmul(eps_list[st][:p, :],
                                 lhsT=xbt[:, 128*st : 128*st+p],
                                 rhs=cmn[:, :],
                                 start=start, stop=(not start))

            def obs_view(st):
                if st < 3:
                    return Bt[:, B_OBL + 512*st : B_OBL + 512*st + 512]
                return A[:, A_OBH + 512*(st-3) : A_OBH + 512*(st-3) + 512]

            def add_st(st):
                p = PS[st]
                esb = epool.tile([128, 512], bf16, tag="etile")
                esb_list[st] = esb
                nc.vector.tensor_add(esb[:p, :], obs_view(st)[:p, :], eps_list[st][:p, :])

            def gram_st(st, first):
                p = PS[st]
                esb = esb_list[st]
                for g in range(4):
                    # start=True zeroes the full 2KB bank row per written
                    # partition, so only the very first matmul may set it.
                    nc.tensor.matmul(gram_ps[:, 128*g : 128*g+128],
                                     lhsT=esb[:p, 128*g : 128*g+128],
                                     rhs=esb[:p, 128*g : 128*g+128],
                                     start=(first and g == 0),
                                     stop=(st == NTS - 1 and g == 3),
                                     skip_group_check=True)

            for st in range(NTS):
                eps_list[st] = eppool.tile([128, 512], f32, tag="epsum",
                                           name=f"eps{st}")

            conv_st(0, start=True)

            # ---- gts bf16 [32, 776]
            gts = wpool.tile([32, PSI], bf16)
            nc.vector.tensor_copy(gts[:, 0:512], gtA[:])
            nc.vector.tensor_copy(gts[:, 512:PSI], gtB[:])

            # ---- gS [128, 776]: group rho = gT shifted right by rho cols
            gsA = ppool.tile([128, 512], f32, tag="ps2k")
            gsB = ppool.tile([128, 264], f32, tag="ps2k")
            for rho in range(4):
                tp = (0, 32 * rho) if rho else None
                nc.tensor.matmul(gsA[32*rho : 32*rho+32, rho:512],
                                 lhsT=identb[:],
                                 rhs=gts[:, 0 : 512-rho],
                                 start=True, stop=True, tile_position=tp)
                nc.tensor.matmul(gsB[32*rho : 32*rho+32, 0:264],
                                 lhsT=identb[:],
                                 rhs=gts[:, 512-rho : PSI-rho],
                                 start=True, stop=True, tile_position=tp)

            conv_st(1, start=True)

            gss = wpool.tile([128, PSI], bf16)
            nc.vector.tensor_copy(gss[:, 0:512], gsA[:])
            nc.scalar.copy(gss[:, 512:PSI], gsB[:])

            # ---- XbT [32, 760]: sum_j D_j g_{s+15-j} via 4 tap-groups of 4
            xbtA = ppool.tile([32, 512], f32, tag="ps2k")
            xbtB = ppool.tile([32, 248], f32, tag="ps2k")
            for jg, j0 in enumerate(J0S):
                nc.tensor.matmul(xbtA[:, 0:512],
                                 lhsT=Bt[:, B_DSTK + 32*jg : B_DSTK + 32*jg+32],
                                 rhs=gss[:, (J-1-j0) : (J-1-j0) + 512],
                                 start=(j0 == 0), stop=(j0 == J0S[-1]))
            for jg, j0 in enumerate(J0S):
                nc.tensor.matmul(xbtB[:, 0:248],
                                 lhsT=Bt[:, B_DSTK + 32*jg : B_DSTK + 32*jg+32],
                                 rhs=gss[:, (J-1-j0) + 512 : (J-1-j0) + 760],
                                 start=(j0 == 0), stop=(j0 == J0S[-1]))

            conv_st(2, start=True)

            xbt = wpool.tile([32, NB], bf16)
            nc.vector.tensor_copy(xbt[:, 0:512], xbtA[:])
            nc.vector.tensor_copy(xbt[:, 512:NB], xbtB[:])

            # ---- early sts: close with xbt term; late sts: open with it
            xcmn_st(0, start=False, xbt=xbt)
            add_st(0)
            xcmn_st(1, start=False, xbt=xbt)
            add_st(1)
            xcmn_st(2, start=False, xbt=xbt)
            add_st(2)
            xcmn_st(3, start=True, xbt=xbt)
            conv_st(3, start=False)
            add_st(3)
            xcmn_st(4, start=True, xbt=xbt)
            conv_st(4, start=False)
            add_st(4)
            gram_st(0, first=True)
            gram_st(1, first=False)
            xcmn_st(5, start=True, xbt=xbt)
            conv_st(5, start=False)
            add_st(5)
            gram_st(2, first=False)
            gram_st(3, first=False)
            gram_st(4, first=False)
            gram_st(5, first=False)

            # ---- Gram PSUM -> SBUF (split DVE/ACT), out on both rings
            ysb = wpool.tile([128, 512], f32)
            nc.vector.tensor_copy(ysb[:, 0:256], gram_ps[:, 0:256])
            nc.scalar.copy(ysb[:, 256:512], gram_ps[:, 256:512])
            nc.sync.dma_start(yout[:, 0:256], ysb[:, 0:256])
            nc.scalar.dma_start(yout[:, 256:512], ysb[:, 256:512])

    _split_multi_waits(nc)
    return nc


def _core_inputs(obs, c, consts):
    """Host-side layout prep for one core: pack bulkA / bulkB / smalls."""
    import ml_dtypes
    bf16 = ml_dtypes.bfloat16
    trilh, gmat, cmn, dstk, identb = consts
    start = W + c * R
    hb = J * B + 1                                      # halo rows + 1
    flat = obs[start - hb : start + R]
    # Oblk tiles: rows [start+16s, +16) for s in [0, 760)
    ob = np.zeros((768, 512), np.float32)
    ob[:NB] = flat[hb : hb + R].reshape(NB, 512)
    obsb = np.ascontiguousarray(
        ob.reshape(6, 128, 512).transpose(1, 0, 2).reshape(128, 6 * 512))
    # panel rows (shifted by -1 obs row): s in [0, 760)
    pm = np.zeros((768, 512), np.float32)
    pm[:NB] = flat[hb - 1 : hb - 1 + R].reshape(NB, 512)
    ptm = pm.reshape(768, 4, 128).transpose(2, 1, 0)    # [128, 4, 768]
    pth = np.zeros((128, 4, 32), np.float32)
    ph = flat[0 : J * B].reshape(J, 512)                # halo panel rows
    pth[:, :, :J] = ph.reshape(J, 4, 128).transpose(2, 1, 0)

    bulka = np.zeros((128, A_COLS), np.float32)
    bulka[:, 0:A_HALO] = gmat.astype(np.float32)
    bulka[:, A_HALO:A_PANS] = pth.reshape(128, 128)
    bulka[:, A_PANS:A_OBH] = ptm.reshape(128, 3072)
    bulka[:, A_OBH:A_COLS] = obsb[:, 1536:3072]

    bulkb = np.zeros((128, B_COLS), np.float32)
    bulkb[:, 0:B_DSTK] = trilh.astype(np.float32)
    bulkb[:, B_DSTK:B_OBL] = dstk.astype(np.float32)
    bulkb[:, B_OBL:B_COLS] = obsb[:, 0:1536]

    smalls = np.zeros((32, S_COLS), np.float32)
    smalls[:, 0:32] = identb.astype(np.float32)
    smalls[:, 32:544] = cmn.astype(np.float32)

    return {"bulka": bulka.astype(bf16), "bulkb": bulkb.astype(bf16),
            "smalls": smalls.astype(bf16)}


def kernel(observations, A, C, K, x0, Psqrt0, _trace=False, _trace_kwargs=None):
    obs = np.ascontiguousarray(observations, np.float32)
    A64 = np.asarray(A, np.float64)
    C64 = np.asarray(C, np.float64)
    K64 = np.asarray(K, np.float64)

    consts = _build_device_consts(A64, C64, K64)
    Y = _host_exact(obs, A64, C64, K64, np.asarray(x0), np.asarray(Psqrt0))

    if "prog" not in _PROG_CACHE:
        _PROG_CACHE["prog"] = build_program()
    nc = _PROG_CACHE["prog"]

    in_maps = [_core_inputs(obs, c, consts) for c in range(NCORES)]

    from concourse.bass_utils import run_bass_kernel_spmd
    kw = dict(_trace_kwargs or {})
    res = run_bass_kernel_spmd(nc, in_maps, list(range(NCORES)), trace=_trace, **kw)

    for c in range(NCORES):
        G = np.asarray(res.results[c]["yout"], np.float64)
        for g in range(4):
            for tau in range(4):
                Y += G[32*tau:32*tau+32, 128*g+32*tau : 128*g+32*tau+32]
    loss = np.linalg.slogdet(Y / T)[1]
    out = np.float32(loss)
    if _trace:
        return out, res
    return out


# revision 6
# speedup vs baseline: 1.1714x; 1.0621x over previous
"""Trainium2 Bass kernel for the differentiable-Kalman-filter loss.

Math: the reference runs a T=100000-step linear recurrence
  x_{i+1} = M x_i + K obs[i-1],  eps_i = obs[i] - C x_{i+1},  M = A - K C
and accumulates yvar = sum outer(eps_i) + decaying P-terms, loss = slogdet(yvar/T).
rho(M) ~ 0.963, so the recurrence has ~400-step memory: eps becomes a truncated
causal convolution of obs.  Each core computes eps for a 12160-row slab via a
two-level blocked conv (B=16 within-block taps as one 512x512 triangular matmul,
block-boundary states from J=16 block-level taps), then accumulates the Gram
E^T E on-chip.  The first W=2720 rows + the tiny P-series are computed exactly
on host in f64 (they need the exact initial transient and cost ~nothing).

Perf layout (v3): input halves of each tensor ride both HWDGE rings in
need-order (rings drain round-robin, so per-ring order alone cannot
prioritize); PE kept busy from ~7us via junk matmuls on a memset tile (HAM
clock gate re-throttles after ~3.4us idle); late eps groups open with the xbt
term so the last tile's ADD->gram tail is short.
"""
import numpy as np

T, N, B, J, W, NCORES = 100000, 32, 16, 16, 2720, 8
R = (T - W) // NCORES       # rows per core = 12160
NB = R // B                 # 760 blocks per core
PSI = NB + J                # 776 panel columns (incl halo)
NTS = 6                     # s-tiles per core
PS = [128, 128, 128, 128, 128, 120]
J0S = [0, 4, 8, 12]

# ringA: gmat | pans kc0,kc1 | trilh kc0,kc1 | obsb st0-2
# ringB: halo | pans kc2,kc3 | trilh kc2,kc3 | dstk | obsb st3-5
A_PANS, A_TRI, A_OBS, A_COLS = 128, 1664, 2688, 4224
B_PANS, B_TRI, B_DSTK, B_OBS, B_COLS = 128, 1664, 2688, 2816, 4352
S_COLS = 544                # smalls: identb(32) | cmn(512)

_PROG_CACHE = {}


def _build_device_consts(A64, C64, K64):
    import ml_dtypes
    bf16 = ml_dtypes.bfloat16
    M = A64 - K64 @ C64
    Mp = [np.eye(N)]
    for _ in range(B + 1):
        Mp.append(M @ Mp[-1])
    H = [C64 @ Mp[k] @ K64 for k in range(B)]
    TrilHneg = np.zeros((512, 512))
    for r in range(B):
        for t in range(r, B):
            TrilHneg[r*N:(r+1)*N, t*N:(t+1)*N] = -H[t - r].T
    Gmat = np.zeros((512, N))
    for r in range(B):
        Gmat[r*N:(r+1)*N, :] = (Mp[B-1-r] @ K64).T
    CMn = np.zeros((N, 512))
    for t in range(B):
        CMn[:, t*N:(t+1)*N] = -(C64 @ Mp[t+1]).T
    MB = Mp[B]
    D = [np.eye(N)]
    for _ in range(J - 1):
        D.append(MB @ D[-1])
    trilh = np.ascontiguousarray(TrilHneg.reshape(4, 128, 512).transpose(1, 0, 2).reshape(128, 2048)).astype(bf16)
    gmat = np.ascontiguousarray(Gmat.reshape(4, 128, N).transpose(1, 0, 2).reshape(128, 128)).astype(bf16)
    cmn = np.ascontiguousarray(CMn).astype(bf16)
    dstk = np.zeros((128, 32 * (J // 4)))
    for jg in range(J // 4):
        for rho in range(4):
            dstk[32*rho:32*rho+32, 32*jg:32*jg+32] = D[4*jg + rho].T
    dstk = dstk.astype(bf16)
    identb = np.eye(32).astype(bf16)
    return trilh, gmat, cmn, dstk, identb


def _host_exact(obs, A64, C64, K64, x0, Psqrt0):
    """f64 exact: P-series + outer(obs0) + eps outers for i < W."""
    obs64 = obs.astype(np.float64)
    M = A64 - K64 @ C64
    Y = np.outer(obs64[0], obs64[0])
    P = Psqrt0.astype(np.float64)
    for _ in range(4000):
        CP = C64 @ P
        Y += CP @ CP.T
        P = M @ P
        if np.abs(P).max() < 1e-16:
            break
    x = x0.astype(np.float64)
    for i in range(W):
        o_prev = obs64[i - 1] if i > 0 else obs64[T - 1]
        x = M @ x + K64 @ o_prev
        eps = obs64[i] - C64 @ x
        Y += np.outer(eps, eps)
    return Y


def _patch_tile_drain():
    """This walrus build allows only one sem wait per Drain; split the
    TileContext tail drain's waits across multiple drain instructions."""
    import concourse.tile as tile
    from concourse.vector_clock import ScopedClock
    if getattr(tile.TileContext, "_kf_drain_patched", False):
        return
    def _drain_and_barrier(self, tick_clock, wait_clock):
        nc = self.nc
        drain_inst = nc.sync.drain()
        wait_clock.add_sem_waits(drain_inst.ins, ScopedClock({None: tick_clock.global_clock}))
        si = drain_inst.ins.sync_info
        waits = list(si.on_wait or [])
        if len(waits) > 1:
            si.on_wait = waits[:1]
            for i in range(1, len(waits)):
                extra = nc.sync.drain()
                esi = extra.ins.sync_info
                if esi is None:
                    extra.ins.sync_info = type(si)(on_wait=waits[i:i+1], on_update=[])
                else:
                    esi.on_wait = waits[i:i+1]
        nc.all_engine_barrier(sem_only=True)
        assert self.sems is not None
        popped = nc._tile_sem_poison_stack.pop()
        assert popped is self._sem_poison
        nc.clear_and_free_semaphores(list(self.sems.allocated().values()))
    tile.TileContext._drain_and_barrier = _drain_and_barrier
    tile.TileContext._kf_drain_patched = True


def _split_multi_waits(nc):
    """This walrus build encodes at most one sem wait per instruction; hoist
    extra waits onto NoOps inserted just before in the same engine stream."""
    import concourse.mybir as mybir
    for func in nc.m.functions:
        for blk in func.blocks:
            insts = blk.instructions
            out, changed = [], False
            for inst in insts:
                si = inst.sync_info
                waits = list(si.on_wait) if si and si.on_wait else []
                if len(waits) > 1:
                    changed = True
                    for k, w in enumerate(waits[:-1]):
                        out.append(mybir.InstNoOp(
                            name=f"{inst.name}-hw{k}", engine=inst.engine,
                            bass_nofuse=True,
                            sync_info=mybir.SyncInfo(on_wait=[w], on_update=[])))
                    si.on_wait = [waits[-1]]
                out.append(inst)
            if changed:
                blk.instructions = out


def build_program(debug=False):
    import concourse.bass as bass
    import concourse.mybir as mybir
    import concourse.tile as tile
    _patch_tile_drain()
    f32 = mybir.dt.float32
    bf16 = mybir.dt.bfloat16

    nc = bass.Bass()
    A_in = nc.declare_dram_parameter("bulka", [128, A_COLS], bf16, isOutput=False)
    B_in = nc.declare_dram_parameter("bulkb", [128, B_COLS], bf16, isOutput=False)
    S_in = nc.declare_dram_parameter("smalls", [32, S_COLS], bf16, isOutput=False)
    yout = nc.declare_dram_parameter("yout", [128, 512], f32, isOutput=True)

    with tile.TileContext(nc) as tc:
        with (
            tc.tile_pool(name="big", bufs=1) as bpool,
            tc.tile_pool(name="work", bufs=1) as wpool,
            tc.tile_pool(name="etile", bufs=6) as epool,
            tc.tile_pool(name="ps2k", bufs=2, space="PSUM") as ppool,
            tc.tile_pool(name="epsum", bufs=4, space="PSUM") as eppool,
            tc.tile_pool(name="gramps", bufs=1, space="PSUM") as gpool,
        ):
            RA = bpool.tile([128, A_COLS], bf16)
            RB = bpool.tile([128, B_COLS], bf16)
            S = wpool.tile([32, S_COLS], bf16)
            warm = wpool.tile([128, 128], bf16)

            # ---- warm tile via memset (no DMA dep), smalls on SWDGE ring
            nc.gpsimd.memset(warm[:], 0.0)
            nc.gpsimd.dma_start(S[:], S_in[:])
            # ---- bulk inputs: each tensor half on each HWDGE ring, need-order
            nc.sync.dma_start(RA[:, 0:A_PANS], A_in[:, 0:A_PANS])
            nc.scalar.dma_start(RB[:, 0:B_PANS], B_in[:, 0:B_PANS])
            nc.sync.dma_start(RA[:, A_PANS:A_TRI], A_in[:, A_PANS:A_TRI])
            nc.scalar.dma_start(RB[:, B_PANS:B_TRI], B_in[:, B_PANS:B_TRI])
            nc.sync.dma_start(RA[:, A_TRI:A_OBS], A_in[:, A_TRI:A_OBS])
            nc.scalar.dma_start(RB[:, B_TRI:B_OBS], B_in[:, B_TRI:B_OBS])
            nc.sync.dma_start(RA[:, A_OBS:A_COLS], A_in[:, A_OBS:A_COLS])
            nc.scalar.dma_start(RB[:, B_OBS:B_COLS], B_in[:, B_OBS:B_COLS])

            gmat = RA[:, 0:128]
            identb = S[:, 0:32]
            cmn = S[:, 32:544]

            def pans(kc, c0, c1):
                if kc < 2:
                    return RA[:, A_PANS + 768*kc + c0 : A_PANS + 768*kc + c1]
                return RB[:, B_PANS + 768*(kc-2) + c0 : B_PANS + 768*(kc-2) + c1]

            def trilh(kc):
                if kc < 2:
                    return RA[:, A_TRI + 512*kc : A_TRI + 512*kc + 512]
                return RB[:, B_TRI + 512*(kc-2) : B_TRI + 512*(kc-2) + 512]

            gram_ps = gpool.tile([128, 512], f32)

            def junk(n):
                # PE keep-warm: HAM un-throttles only under sustained activity
                for _ in range(n):
                    nc.tensor.matmul(gram_ps[:, 0:128], lhsT=warm[:], rhs=warm[:],
                                     start=True, stop=True, skip_group_check=True)

            junk(16)

            # ---- gT [32, 776]: halo states + main panel states (split 512|264)
            gtA = ppool.tile([32, 512], f32, tag="ps2k")
            gtB = ppool.tile([32, 264], f32, tag="ps2k")
            for kc in range(4):
                nc.tensor.matmul(gtA[:, 0:J],
                                 lhsT=gmat[:, 32*kc : 32*kc+32],
                                 rhs=RB[:, 32*kc : 32*kc + J],
                                 start=(kc == 0), stop=False)
            junk(28)
            for kc in range(4):
                nc.tensor.matmul(gtA[:, J : 512],
                                 lhsT=gmat[:, 32*kc : 32*kc+32],
                                 rhs=pans(kc, 0, 512 - J),
                                 start=False, stop=(kc == 3))
            for kc in range(4):
                nc.tensor.matmul(gtB[:, 0:264],
                                 lhsT=gmat[:, 32*kc : 32*kc+32],
                                 rhs=pans(kc, 512 - J, 760),
                                 start=(kc == 0), stop=(kc == 3))

            eps_list = [None] * NTS
            esb_list = [None] * NTS
            for st in range(NTS):
                eps_list[st] = eppool.tile([128, 512], f32, tag="epsum",
                                           name=f"eps{st}")

            def conv_st(st, start):
                p = PS[st]
                eps_ps = eps_list[st]
                for kc in range(4):
                    nc.tensor.matmul(eps_ps[:p, :],
                                     lhsT=pans(kc, 128*st, 128*st + p),
                                     rhs=trilh(kc),
                                     start=(start and kc == 0),
                                     stop=(not start and kc == 3))

            def xcmn_st(st, start, xbt):
                p = PS[st]
                nc.tensor.matmul(eps_list[st][:p, :],
                                 lhsT=xbt[:, 128*st : 128*st+p],
                                 rhs=cmn[:, :],
                                 start=start, stop=(not start))

            def obs_view(st):
                if st < 3:
                    return RA[:, A_OBS + 512*st : A_OBS + 512*st + 512]
                return RB[:, B_OBS + 512*(st-3) : B_OBS + 512*(st-3) + 512]

            def add_st(st, c0=0, c1=512):
                p = PS[st]
                if esb_list[st] is None:
                    esb_list[st] = epool.tile([128, 512], bf16, tag="etile",
                                              name=f"esb{st}")
                esb = esb_list[st]
                nc.vector.tensor_add(esb[:p, c0:c1], obs_view(st)[:p, c0:c1],
                                     eps_list[st][:p, c0:c1])

            def gram_st(st, first, gs=(0, 1, 2, 3)):
                p = PS[st]
                esb = esb_list[st]
                for g in gs:
                    # start=True zeroes the full 2KB bank row per written
                    # partition, so only the very first matmul may set it.
                    nc.tensor.matmul(gram_ps[:, 128*g : 128*g+128],
                                     lhsT=esb[:p, 128*g : 128*g+128],
                                     rhs=esb[:p, 128*g : 128*g+128],
                                     start=(first and g == 0),
                                     stop=(st == NTS - 1 and g == 3),
                                     skip_group_check=True)

            conv_st(0, start=True)

            # ---- gts bf16 [32, 776]
            gts = wpool.tile([32, PSI], bf16)
            nc.vector.tensor_copy(gts[:, 0:512], gtA[:])
            nc.vector.tensor_copy(gts[:, 512:PSI], gtB[:])

            # ---- gS [128, 776]: group rho = gT shifted right by rho cols
            gsA = ppool.tile([128, 512], f32, tag="ps2k")
            gsB = ppool.tile([128, 264], f32, tag="ps2k")
            for rho in range(4):
                tp = (0, 32 * rho) if rho else None
                nc.tensor.matmul(gsA[32*rho : 32*rho+32, rho:512],
                                 lhsT=identb[:],
                                 rhs=gts[:, 0 : 512-rho],
                                 start=True, stop=True, tile_position=tp)
                nc.tensor.matmul(gsB[32*rho : 32*rho+32, 0:264],
                                 lhsT=identb[:],
                                 rhs=gts[:, 512-rho : PSI-rho],
                                 start=True, stop=True, tile_position=tp)

            conv_st(1, start=True)

            gss = wpool.tile([128, PSI], bf16)
            nc.vector.tensor_copy(gss[:, 0:512], gsA[:])
            nc.scalar.copy(gss[:, 512:PSI], gsB[:])

            # ---- XbT [32, 760]: sum_j D_j g_{s+15-j} via 4 tap-groups of 4
            xbtA = ppool.tile([32, 512], f32, tag="ps2k")
            xbtB = ppool.tile([32, 248], f32, tag="ps2k")
            for jg, j0 in enumerate(J0S):
                nc.tensor.matmul(xbtA[:, 0:512],
                                 lhsT=RB[:, B_DSTK + 32*jg : B_DSTK + 32*jg+32],
                                 rhs=gss[:, (J-1-j0) : (J-1-j0) + 512],
                                 start=(j0 == 0), stop=(j0 == J0S[-1]))
            for jg, j0 in enumerate(J0S):
                nc.tensor.matmul(xbtB[:, 0:248],
                                 lhsT=RB[:, B_DSTK + 32*jg : B_DSTK + 32*jg+32],
                                 rhs=gss[:, (J-1-j0) + 512 : (J-1-j0) + 760],
                                 start=(j0 == 0), stop=(j0 == J0S[-1]))

            conv_st(2, start=True)

            xbt = wpool.tile([32, NB], bf16)
            nc.vector.tensor_copy(xbt[:, 0:512], xbtA[:])
            nc.vector.tensor_copy(xbt[:, 512:NB], xbtB[:])

            # ---- early sts: close with xbt term; late sts: open with it
            xcmn_st(0, start=False, xbt=xbt)
            add_st(0)
            xcmn_st(1, start=False, xbt=xbt)
            add_st(1)
            xcmn_st(2, start=False, xbt=xbt)
            add_st(2)
            xcmn_st(3, start=True, xbt=xbt)
            conv_st(3, start=False)
            add_st(3)
            xcmn_st(4, start=True, xbt=xbt)
            conv_st(4, start=False)
            add_st(4)
            gram_st(0, first=True)
            gram_st(1, first=False)
            xcmn_st(5, start=True, xbt=xbt)
            conv_st(5, start=False)
            gram_st(2, first=False)
            gram_st(3, first=False)
            # last tile: split ADD so gram g0/g1 overlap the second half
            add_st(5, 0, 256)
            gram_st(4, first=False)
            add_st(5, 256, 512)
            gram_st(5, first=False, gs=(0, 1))
            gram_st(5, first=False, gs=(2, 3))

            # ---- Gram PSUM -> SBUF (split DVE/ACT), single output DMA
            ysb = wpool.tile([128, 512], f32)
            nc.vector.tensor_copy(ysb[:, 0:256], gram_ps[:, 0:256])
            nc.scalar.copy(ysb[:, 256:512], gram_ps[:, 256:512])
            nc.sync.dma_start(yout[:], ysb[:])

    _split_multi_waits(nc)
    return nc


def _core_inputs(obs, c, consts):
    """Host-side layout prep for one core: pack ringA / ringB / smalls."""
    import ml_dtypes
    bf16 = ml_dtypes.bfloat16
    trilh, gmat, cmn, dstk, identb = consts
    start = W + c * R
    hb = J * B + 1                                      # halo rows + 1
    flat = obs[start - hb : start + R]
    # Oblk tiles: rows [start+16s, +16) for s in [0, 760)
    ob = np.zeros((768, 512), np.float32)
    ob[:NB] = flat[hb : hb + R].reshape(NB, 512)
    obsb = np.ascontiguousarray(
        ob.reshape(6, 128, 512).transpose(1, 0, 2).reshape(128, 6 * 512))
    # panel rows (shifted by -1 obs row): s in [0, 760)
    pm = np.zeros((768, 512), np.float32)
    pm[:NB] = flat[hb - 1 : hb - 1 + R].reshape(NB, 512)
    ptm = pm.reshape(768, 4, 128).transpose(2, 1, 0)    # [128, 4, 768]
    pth = np.zeros((128, 4, 32), np.float32)
    ph = flat[0 : J * B].reshape(J, 512)                # halo panel rows
    pth[:, :, :J] = ph.reshape(J, 4, 128).transpose(2, 1, 0)

    trilh32 = trilh.astype(np.float32)
    bulka = np.zeros((128, A_COLS), np.float32)
    bulka[:, 0:128] = gmat.astype(np.float32)
    bulka[:, A_PANS:A_TRI] = ptm[:, 0:2, :].reshape(128, 1536)
    bulka[:, A_TRI:A_OBS] = trilh32[:, 0:1024]
    bulka[:, A_OBS:A_COLS] = obsb[:, 0:1536]

    bulkb = np.zeros((128, B_COLS), np.float32)
    bulkb[:, 0:128] = pth.reshape(128, 128)
    bulkb[:, B_PANS:B_TRI] = ptm[:, 2:4, :].reshape(128, 1536)
    bulkb[:, B_TRI:B_DSTK] = trilh32[:, 1024:2048]
    bulkb[:, B_DSTK:B_OBS] = dstk.astype(np.float32)
    bulkb[:, B_OBS:B_COLS] = obsb[:, 1536:3072]

    smalls = np.zeros((32, S_COLS), np.float32)
    smalls[:, 0:32] = identb.astype(np.float32)
    smalls[:, 32:544] = cmn.astype(np.float32)

    return {"bulka": bulka.astype(bf16), "bulkb": bulkb.astype(bf16),
            "smalls": smalls.astype(bf16)}


def kernel(observations, A, C, K, x0, Psqrt0, _trace=False, _trace_kwargs=None):
    obs = np.ascontiguousarray(observations, np.float32)
    A64 = np.asarray(A, np.float64)
    C64 = np.asarray(C, np.float64)
    K64 = np.asarray(K, np.float64)

    consts = _build_device_consts(A64, C64, K64)
    Y = _host_exact(obs, A64, C64, K64, np.asarray(x0), np.asarray(Psqrt0))

    if "prog" not in _PROG_CACHE:
        _PROG_CACHE["prog"] = build_program()
    nc = _PROG_CACHE["prog"]

    in_maps = [_core_inputs(obs, c, consts) for c in range(NCORES)]

    from concourse.bass_utils import run_bass_kernel_spmd
    kw = dict(_trace_kwargs or {})
    res = run_bass_kernel_spmd(nc, in_maps, list(range(NCORES)), trace=_trace, **kw)

    for c in range(NCORES):
        G = np.asarray(res.results[c]["yout"], np.float64)
        for g in range(4):
            for tau in range(4):
                Y += G[32*tau:32*tau+32, 128*g+32*tau : 128*g+32*tau+32]
    loss = np.linalg.slogdet(Y / T)[1]
    out = np.float32(loss)
    if _trace:
        return out, res
    return out


# revision 7
# speedup vs baseline: 1.1730x; 1.0013x over previous
"""Trainium2 Bass kernel for the differentiable-Kalman-filter loss.

Math: the reference runs a T=100000-step linear recurrence
  x_{i+1} = M x_i + K obs[i-1],  eps_i = obs[i] - C x_{i+1},  M = A - K C
and accumulates yvar = sum outer(eps_i) + decaying P-terms, loss = slogdet(yvar/T).
rho(M) ~ 0.963, so the recurrence has ~400-step memory: eps becomes a truncated
causal convolution of obs.  Each core computes eps for a 12160-row slab via a
two-level blocked conv (B=16 within-block taps as one 512x512 triangular matmul,
block-boundary states from J=16 block-level taps), then accumulates the Gram
E^T E on-chip.  The first W=2720 rows + the tiny P-series are computed exactly
on host in f64 (they need the exact initial transient and cost ~nothing).

Perf layout (v3): input halves of each tensor ride both HWDGE rings in
need-order (rings drain round-robin, so per-ring order alone cannot
prioritize); PE kept busy from ~7us via junk matmuls on a memset tile (HAM
clock gate re-throttles after ~3.4us idle); late eps groups open with the xbt
term so the last tile's ADD->gram tail is short.
"""
import numpy as np

T, N, B, J, W, NCORES = 100000, 32, 16, 16, 2720, 8
R = (T - W) // NCORES       # rows per core = 12160
NB = R // B                 # 760 blocks per core
PSI = NB + J                # 776 panel columns (incl halo)
NTS = 6                     # s-tiles per core
PS = [128, 128, 128, 128, 128, 120]
J0S = [0, 4, 8, 12]

# ringA: gmat | pans kc0,kc1 | trilh kc0,kc1 | obsb st0-2
# ringB: halo | pans kc2,kc3 | trilh kc2,kc3 | dstk | obsb st3-5
A_PANS, A_TRI, A_OBS, A_COLS = 128, 1664, 2688, 4224
B_PANS, B_TRI, B_DSTK, B_OBS, B_COLS = 128, 1664, 2688, 2816, 4352
S_COLS = 544                # smalls: identb(32) | cmn(512)

_PROG_CACHE = {}


def _build_device_consts(A64, C64, K64):
    import ml_dtypes
    bf16 = ml_dtypes.bfloat16
    M = A64 - K64 @ C64
    Mp = [np.eye(N)]
    for _ in range(B + 1):
        Mp.append(M @ Mp[-1])
    H = [C64 @ Mp[k] @ K64 for k in range(B)]
    TrilHneg = np.zeros((512, 512))
    for r in range(B):
        for t in range(r, B):
            TrilHneg[r*N:(r+1)*N, t*N:(t+1)*N] = -H[t - r].T
    Gmat = np.zeros((512, N))
    for r in range(B):
        Gmat[r*N:(r+1)*N, :] = (Mp[B-1-r] @ K64).T
    CMn = np.zeros((N, 512))
    for t in range(B):
        CMn[:, t*N:(t+1)*N] = -(C64 @ Mp[t+1]).T
    MB = Mp[B]
    D = [np.eye(N)]
    for _ in range(J - 1):
        D.append(MB @ D[-1])
    trilh = np.ascontiguousarray(TrilHneg.reshape(4, 128, 512).transpose(1, 0, 2).reshape(128, 2048)).astype(bf16)
    gmat = np.ascontiguousarray(Gmat.reshape(4, 128, N).transpose(1, 0, 2).reshape(128, 128)).astype(bf16)
    cmn = np.ascontiguousarray(CMn).astype(bf16)
    dstk = np.zeros((128, 32 * (J // 4)))
    for jg in range(J // 4):
        for rho in range(4):
            dstk[32*rho:32*rho+32, 32*jg:32*jg+32] = D[4*jg + rho].T
    dstk = dstk.astype(bf16)
    identb = np.eye(32).astype(bf16)
    return trilh, gmat, cmn, dstk, identb


def _host_exact(obs, A64, C64, K64, x0, Psqrt0):
    """f64 exact: P-series + outer(obs0) + eps outers for i < W."""
    obs64 = obs.astype(np.float64)
    M = A64 - K64 @ C64
    Y = np.outer(obs64[0], obs64[0])
    P = Psqrt0.astype(np.float64)
    for _ in range(4000):
        CP = C64 @ P
        Y += CP @ CP.T
        P = M @ P
        if np.abs(P).max() < 1e-16:
            break
    x = x0.astype(np.float64)
    for i in range(W):
        o_prev = obs64[i - 1] if i > 0 else obs64[T - 1]
        x = M @ x + K64 @ o_prev
        eps = obs64[i] - C64 @ x
        Y += np.outer(eps, eps)
    return Y


def _patch_tile_drain():
    """This walrus build allows only one sem wait per Drain; split the
    TileContext tail drain's waits across multiple drain instructions."""
    import concourse.tile as tile
    from concourse.vector_clock import ScopedClock
    if getattr(tile.TileContext, "_kf_drain_patched", False):
        return
    def _drain_and_barrier(self, tick_clock, wait_clock):
        nc = self.nc
        drain_inst = nc.sync.drain()
        wait_clock.add_sem_waits(drain_inst.ins, ScopedClock({None: tick_clock.global_clock}))
        si = drain_inst.ins.sync_info
        waits = list(si.on_wait or [])
        if len(waits) > 1:
            si.on_wait = waits[:1]
            for i in range(1, len(waits)):
                extra = nc.sync.drain()
                esi = extra.ins.sync_info
                if esi is None:
                    extra.ins.sync_info = type(si)(on_wait=waits[i:i+1], on_update=[])
                else:
                    esi.on_wait = waits[i:i+1]
        nc.all_engine_barrier(sem_only=True)
        assert self.sems is not None
        popped = nc._tile_sem_poison_stack.pop()
        assert popped is self._sem_poison
        nc.clear_and_free_semaphores(list(self.sems.allocated().values()))
    tile.TileContext._drain_and_barrier = _drain_and_barrier
    tile.TileContext._kf_drain_patched = True


def _split_multi_waits(nc):
    """This walrus build encodes at most one sem wait per instruction; hoist
    extra waits onto NoOps inserted just before in the same engine stream."""
    import concourse.mybir as mybir
    for func in nc.m.functions:
        for blk in func.blocks:
            insts = blk.instructions
            out, changed = [], False
            for inst in insts:
                si = inst.sync_info
                waits = list(si.on_wait) if si and si.on_wait else []
                if len(waits) > 1:
                    changed = True
                    for k, w in enumerate(waits[:-1]):
                        out.append(mybir.InstNoOp(
                            name=f"{inst.name}-hw{k}", engine=inst.engine,
                            bass_nofuse=True,
                            sync_info=mybir.SyncInfo(on_wait=[w], on_update=[])))
                    si.on_wait = [waits[-1]]
                out.append(inst)
            if changed:
                blk.instructions = out


def build_program(debug=False):
    import concourse.bass as bass
    import concourse.mybir as mybir
    import concourse.tile as tile
    _patch_tile_drain()
    f32 = mybir.dt.float32
    bf16 = mybir.dt.bfloat16

    nc = bass.Bass()
    A_in = nc.declare_dram_parameter("bulka", [128, A_COLS], bf16, isOutput=False)
    B_in = nc.declare_dram_parameter("bulkb", [128, B_COLS], bf16, isOutput=False)
    S_in = nc.declare_dram_parameter("smalls", [32, S_COLS], bf16, isOutput=False)
    yout = nc.declare_dram_parameter("yout", [128, 512], f32, isOutput=True)

    with tile.TileContext(nc) as tc:
        with (
            tc.tile_pool(name="big", bufs=1) as bpool,
            tc.tile_pool(name="work", bufs=1) as wpool,
            tc.tile_pool(name="etile", bufs=6) as epool,
            tc.tile_pool(name="ps2k", bufs=2, space="PSUM") as ppool,
            tc.tile_pool(name="epsum", bufs=4, space="PSUM") as eppool,
            tc.tile_pool(name="gramps", bufs=2, space="PSUM") as gpool,
        ):
            RA = bpool.tile([128, A_COLS], bf16)
            RB = bpool.tile([128, B_COLS], bf16)
            S = wpool.tile([32, S_COLS], bf16)
            warm = wpool.tile([128, 128], bf16)

            # ---- warm tile via memset (no DMA dep), smalls on SWDGE ring
            nc.gpsimd.memset(warm[:], 0.0)
            nc.gpsimd.dma_start(S[:], S_in[:])
            # ---- bulk inputs: each tensor half on each HWDGE ring, need-order
            nc.sync.dma_start(RA[:, 0:A_PANS], A_in[:, 0:A_PANS])
            nc.scalar.dma_start(RB[:, 0:B_PANS], B_in[:, 0:B_PANS])
            nc.sync.dma_start(RA[:, A_PANS:A_TRI], A_in[:, A_PANS:A_TRI])
            nc.scalar.dma_start(RB[:, B_PANS:B_TRI], B_in[:, B_PANS:B_TRI])
            nc.sync.dma_start(RA[:, A_TRI:A_OBS], A_in[:, A_TRI:A_OBS])
            nc.scalar.dma_start(RB[:, B_TRI:B_OBS], B_in[:, B_TRI:B_OBS])
            nc.sync.dma_start(RA[:, A_OBS:A_COLS], A_in[:, A_OBS:A_COLS])
            nc.scalar.dma_start(RB[:, B_OBS:B_COLS], B_in[:, B_OBS:B_COLS])

            gmat = RA[:, 0:128]
            identb = S[:, 0:32]
            cmn = S[:, 32:544]

            def pans(kc, c0, c1):
                if kc < 2:
                    return RA[:, A_PANS + 768*kc + c0 : A_PANS + 768*kc + c1]
                return RB[:, B_PANS + 768*(kc-2) + c0 : B_PANS + 768*(kc-2) + c1]

            def trilh(kc):
                if kc < 2:
                    return RA[:, A_TRI + 512*kc : A_TRI + 512*kc + 512]
                return RB[:, B_TRI + 512*(kc-2) : B_TRI + 512*(kc-2) + 512]

            gramA = gpool.tile([128, 256], f32, tag="gramps")
            gramB = gpool.tile([128, 256], f32, tag="gramps")

            def junk(n):
                # PE keep-warm: HAM un-throttles only under sustained activity
                for _ in range(n):
                    nc.tensor.matmul(gramA[:, 0:128], lhsT=warm[:], rhs=warm[:],
                                     start=True, stop=True, skip_group_check=True)

            junk(16)

            # ---- gT [32, 776]: halo states + main panel states (split 512|264)
            gtA = ppool.tile([32, 512], f32, tag="ps2k")
            gtB = ppool.tile([32, 264], f32, tag="ps2k")
            for kc in range(4):
                nc.tensor.matmul(gtA[:, 0:J],
                                 lhsT=gmat[:, 32*kc : 32*kc+32],
                                 rhs=RB[:, 32*kc : 32*kc + J],
                                 start=(kc == 0), stop=False)
            junk(28)
            for kc in range(4):
                nc.tensor.matmul(gtA[:, J : 512],
                                 lhsT=gmat[:, 32*kc : 32*kc+32],
                                 rhs=pans(kc, 0, 512 - J),
                                 start=False, stop=(kc == 3))
                nc.tensor.matmul(gtB[:, 0:264],
                                 lhsT=gmat[:, 32*kc : 32*kc+32],
                                 rhs=pans(kc, 512 - J, 760),
                                 start=(kc == 0), stop=(kc == 3))

            eps_list = [None] * NTS
            esb_list = [None] * NTS
            for st in range(NTS):
                eps_list[st] = eppool.tile([128, 512], f32, tag="epsum",
                                           name=f"eps{st}")

            def conv_st(st, start):
                p = PS[st]
                eps_ps = eps_list[st]
                for kc in range(4):
                    nc.tensor.matmul(eps_ps[:p, :],
                                     lhsT=pans(kc, 128*st, 128*st + p),
                                     rhs=trilh(kc),
                                     start=(start and kc == 0),
                                     stop=(not start and kc == 3))

            def xcmn_st(st, start, xbt):
                p = PS[st]
                nc.tensor.matmul(eps_list[st][:p, :],
                                 lhsT=xbt[:, 128*st : 128*st+p],
                                 rhs=cmn[:, :],
                                 start=start, stop=(not start))

            def obs_view(st):
                if st < 3:
                    return RA[:, A_OBS + 512*st : A_OBS + 512*st + 512]
                return RB[:, B_OBS + 512*(st-3) : B_OBS + 512*(st-3) + 512]

            def add_st(st, c0=0, c1=512):
                p = PS[st]
                if esb_list[st] is None:
                    esb_list[st] = epool.tile([128, 512], bf16, tag="etile",
                                              name=f"esb{st}")
                esb = esb_list[st]
                nc.vector.tensor_add(esb[:p, c0:c1], obs_view(st)[:p, c0:c1],
                                     eps_list[st][:p, c0:c1])

            def gram_st(st, first, gs=(0, 1, 2, 3)):
                p = PS[st]
                esb = esb_list[st]
                for g in gs:
                    bank = gramA if g < 2 else gramB
                    # start=True zeroes the full 2KB bank row per written
                    # partition, so only the first matmul per bank may set it.
                    nc.tensor.matmul(bank[:, 128*(g % 2) : 128*(g % 2)+128],
                                     lhsT=esb[:p, 128*g : 128*g+128],
                                     rhs=esb[:p, 128*g : 128*g+128],
                                     start=(first and g in (0, 2)),
                                     stop=(st == NTS - 1 and g % 2 == 1),
                                     skip_group_check=True)

            conv_st(0, start=True)

            # ---- gts bf16 [32, 776]
            gts = wpool.tile([32, PSI], bf16)
            nc.vector.tensor_copy(gts[:, 0:512], gtA[:])
            nc.vector.tensor_copy(gts[:, 512:PSI], gtB[:])

            # ---- gS [128, 776]: group rho = gT shifted right by rho cols
            gsA = ppool.tile([128, 512], f32, tag="ps2k")
            gsB = ppool.tile([128, 264], f32, tag="ps2k")
            for rho in range(4):
                tp = (0, 32 * rho) if rho else None
                nc.tensor.matmul(gsA[32*rho : 32*rho+32, rho:512],
                                 lhsT=identb[:],
                                 rhs=gts[:, 0 : 512-rho],
                                 start=True, stop=True, tile_position=tp)
                nc.tensor.matmul(gsB[32*rho : 32*rho+32, 0:264],
                                 lhsT=identb[:],
                                 rhs=gts[:, 512-rho : PSI-rho],
                                 start=True, stop=True, tile_position=tp)

            conv_st(1, start=True)

            gss = wpool.tile([128, PSI], bf16)
            nc.vector.tensor_copy(gss[:, 0:512], gsA[:])
            nc.scalar.copy(gss[:, 512:PSI], gsB[:])

            # ---- XbT [32, 760]: sum_j D_j g_{s+15-j} via 4 tap-groups of 4
            xbtA = ppool.tile([32, 512], f32, tag="ps2k")
            xbtB = ppool.tile([32, 248], f32, tag="ps2k")
            for jg, j0 in enumerate(J0S):
                nc.tensor.matmul(xbtA[:, 0:512],
                                 lhsT=RB[:, B_DSTK + 32*jg : B_DSTK + 32*jg+32],
                                 rhs=gss[:, (J-1-j0) : (J-1-j0) + 512],
                                 start=(j0 == 0), stop=(j0 == J0S[-1]))
                nc.tensor.matmul(xbtB[:, 0:248],
                                 lhsT=RB[:, B_DSTK + 32*jg : B_DSTK + 32*jg+32],
                                 rhs=gss[:, (J-1-j0) + 512 : (J-1-j0) + 760],
                                 start=(j0 == 0), stop=(j0 == J0S[-1]))

            conv_st(2, start=True)

            xbt = wpool.tile([32, NB], bf16)
            nc.vector.tensor_copy(xbt[:, 0:512], xbtA[:])
            nc.vector.tensor_copy(xbt[:, 512:NB], xbtB[:])

            # ---- early sts: close with xbt term; late sts: open with it
            xcmn_st(0, start=False, xbt=xbt)
            add_st(0)
            xcmn_st(1, start=False, xbt=xbt)
            add_st(1)
            xcmn_st(2, start=False, xbt=xbt)
            add_st(2)
            xcmn_st(3, start=True, xbt=xbt)
            conv_st(3, start=False)
            add_st(3)
            xcmn_st(4, start=True, xbt=xbt)
            conv_st(4, start=False)
            add_st(4)
            gram_st(0, first=True)
            gram_st(1, first=False)
            xcmn_st(5, start=True, xbt=xbt)
            conv_st(5, start=False)
            gram_st(2, first=False)
            gram_st(3, first=False)
            # last tile: split ADD so each gram bank closes early and its
            # copy+DMA overlaps the other half's tail
            ysb = wpool.tile([128, 512], f32)
            add_st(5, 0, 256)
            gram_st(4, first=False)
            add_st(5, 256, 512)
            gram_st(5, first=False, gs=(0, 1))
            nc.vector.tensor_copy(ysb[:, 0:256], gramA[:])
            nc.sync.dma_start(yout[:, 0:256], ysb[:, 0:256])
            gram_st(5, first=False, gs=(2, 3))
            nc.scalar.copy(ysb[:, 256:512], gramB[:])
            nc.scalar.dma_start(yout[:, 256:512], ysb[:, 256:512])

    _split_multi_waits(nc)
    return nc


def _core_inputs(obs, c, consts):
    """Host-side layout prep for one core: pack ringA / ringB / smalls."""
    import ml_dtypes
    bf16 = ml_dtypes.bfloat16
    trilh, gmat, cmn, dstk, identb = consts
    start = W + c * R
    hb = J * B + 1                                      # halo rows + 1
    flat = obs[start - hb : start + R]
    # Oblk tiles: rows [start+16s, +16) for s in [0, 760)
    ob = np.zeros((768, 512), np.float32)
    ob[:NB] = flat[hb : hb + R].reshape(NB, 512)
    obsb = np.ascontiguousarray(
        ob.reshape(6, 128, 512).transpose(1, 0, 2).reshape(128, 6 * 512))
    # panel rows (shifted by -1 obs row): s in [0, 760)
    pm = np.zeros((768, 512), np.float32)
    pm[:NB] = flat[hb - 1 : hb - 1 + R].reshape(NB, 512)
    ptm = pm.reshape(768, 4, 128).transpose(2, 1, 0)    # [128, 4, 768]
    pth = np.zeros((128, 4, 32), np.float32)
    ph = flat[0 : J * B].reshape(J, 512)                # halo panel rows
    pth[:, :, :J] = ph.reshape(J, 4, 128).transpose(2, 1, 0)

    trilh32 = trilh.astype(np.float32)
    bulka = np.zeros((128, A_COLS), np.float32)
    bulka[:, 0:128] = gmat.astype(np.float32)
    bulka[:, A_PANS:A_TRI] = ptm[:, 0:2, :].reshape(128, 1536)
    bulka[:, A_TRI:A_OBS] = trilh32[:, 0:1024]
    bulka[:, A_OBS:A_COLS] = obsb[:, 0:1536]

    bulkb = np.zeros((128, B_COLS), np.float32)
    bulkb[:, 0:128] = pth.reshape(128, 128)
    bulkb[:, B_PANS:B_TRI] = ptm[:, 2:4, :].reshape(128, 1536)
    bulkb[:, B_TRI:B_DSTK] = trilh32[:, 1024:2048]
    bulkb[:, B_DSTK:B_OBS] = dstk.astype(np.float32)
    bulkb[:, B_OBS:B_COLS] = obsb[:, 1536:3072]

    smalls = np.zeros((32, S_COLS), np.float32)
    smalls[:, 0:32] = identb.astype(np.float32)
    smalls[:, 32:544] = cmn.astype(np.float32)

    return {"bulka": bulka.astype(bf16), "bulkb": bulkb.astype(bf16),
            "smalls": smalls.astype(bf16)}


def kernel(observations, A, C, K, x0, Psqrt0, _trace=False, _trace_kwargs=None):
    obs = np.ascontiguousarray(observations, np.float32)
    A64 = np.asarray(A, np.float64)
    C64 = np.asarray(C, np.float64)
    K64 = np.asarray(K, np.float64)

    consts = _build_device_consts(A64, C64, K64)
    Y = _host_exact(obs, A64, C64, K64, np.asarray(x0), np.asarray(Psqrt0))

    if "prog" not in _PROG_CACHE:
        _PROG_CACHE["prog"] = build_program()
    nc = _PROG_CACHE["prog"]

    in_maps = [_core_inputs(obs, c, consts) for c in range(NCORES)]

    from concourse.bass_utils import run_bass_kernel_spmd
    kw = dict(_trace_kwargs or {})
    res = run_bass_kernel_spmd(nc, in_maps, list(range(NCORES)), trace=_trace, **kw)

    for c in range(NCORES):
        G = np.asarray(res.results[c]["yout"], np.float64)
        for g in range(4):
            for tau in range(4):
                Y += G[32*tau:32*tau+32, 128*g+32*tau : 128*g+32*tau+32]
    loss = np.linalg.slogdet(Y / T)[1]
    out = np.float32(loss)
    if _trace:
        return out, res
    return out
